# revision 68
# baseline (speedup 1.0000x reference)
"""Trainium2 Bass kernel for LoRA-augmented causal attention.

Reference computation (per nn_Attention_31688268710508):
  x:(B,S,D) -> q/k/v = x@W* + broadcast LoRA + shared head-offset LoRA,
  RoPE(q,k), causal softmax attention per (b,head), out-proj with wo.

Strategy (8 NeuronCores, tensor-parallel over heads):
  * All rank-8 LoRA terms folded into effective projection weights on the
    host; softmax 1/sqrt(HD) folded into Wq; RoPE pair permutation folded
    into Wq/Wk columns so RoPE is a half-partition rotation; W panels
    pre-transposed on the host so their DMAs are contiguous.
  * fp16 operands on the PE (fp32 PSUM); P/V in bf16; the causal mask is
    a 0/1 multiply after exp (bf16 has range for unmasked exp), and the
    fully-masked leading columns of diagonal blocks are trimmed from the
    scores/PV matmuls.
  * Fine-grained weave: proj 512-col sections interleave with attention
    q-tile steps at one-section lag so the PE never waits on exp/RoPE
    latency; 4 PSUM slots decouple proj matmuls from RoPE evacuation;
    softmax denominators accumulate via DVE/GpSimd adds + one ones-matmul.
  * h-major segment order: head 0's AllToAll fires at the 50% mark fully
    hidden; head 1's runs at the tail split into row-halves (small
    collectives are several times cheaper), covered by head 0's
    out-projection, with head 1's out-projection gated per row-half.
  * HWDGE queues are FIFO and block head-of-line on gated DMAs, so
    collective-gated loads are emitted only at points where nothing
    later on that queue is needed sooner.
"""

import math
import os
import sys
from contextlib import ExitStack

import ml_dtypes
import numpy as np

for _p in ("/opt/trn_rl_repo", "/root/.axon_site/_ro/trn_rl_repo"):
    if os.path.isdir(_p) and _p not in sys.path:
        sys.path.insert(0, _p)

import concourse.bass as bass  # noqa: E402
import concourse.mybir as mybir  # noqa: E402
import concourse.tile as tile  # noqa: E402
from concourse import bacc  # noqa: E402
from concourse.masks import make_identity  # noqa: E402

F32 = mybir.dt.float32
F16 = mybir.dt.float16
BF16 = mybir.dt.bfloat16
EXP = mybir.ActivationFunctionType.Exp


class Cfg:
    def __init__(self, B=2, S=2048, D=2048, HEADS=16, NCORES=8):
        self.B, self.S, self.D, self.NCORES = B, S, D, NCORES
        self.HD = 128
        self.HPC = HEADS // NCORES          # heads per core
        self.BS = B * S
        self.RPC = self.BS // NCORES        # output rows per core
        self.ST = 512                       # q tile / proj col tile
        self.SG = 512                       # x^T slab width
        self.NDB = D // 128                 # contraction blocks
        self.NHB = (HEADS * self.HD) // 128  # out-proj contraction blocks
        self.NQT = S // self.ST             # q tiles per (b,h)
        self.SPS = self.SG // self.ST       # 512-sections per slab
        assert self.HD == 128 and S % self.ST == 0 and self.RPC == self.ST


class _Emitter:
    """Holds all tiles/pools and emits proj/attention/outproj units."""

    def __init__(self, nc, tc, cfg, drams):
        self.nc, self.tc, self.cfg = nc, tc, cfg
        self.d = drams
        self.slab_cache = {}

    # ---- scope A (whole kernel) pools ----
    def open_A(self, ctx):
        tc, cfg = self.tc, self.cfg
        constp = ctx.enter_context(tc.tile_pool(name="const", bufs=1))
        self.ident = constp.tile([128, 128], BF16)
        make_identity(self.nc, self.ident)
        self.ones_sb = constp.tile([128, 128], BF16, name="ones")
        self.nc.vector.memset(self.ones_sb, 1.0)
        self.band_sb = constp.tile([128, 2 * cfg.ST - 128], BF16, name="band")

        qkv = ctx.enter_context(tc.tile_pool(name="qkv", bufs=1))
        self.QT = qkv.tile([128, cfg.HPC, cfg.BS], F16, name="QT")
        self.KT = qkv.tile([128, cfg.HPC, cfg.BS], F16, name="KT")
        self.Vhat = qkv.tile([128, cfg.HPC, cfg.B, cfg.S // 128, 128], BF16,
                             name="Vhat")

        self.ptp = ctx.enter_context(tc.tile_pool(name="ptile", bufs=4))
        self.pap = ctx.enter_context(tc.tile_pool(name="pacc", bufs=2))
        self.aotp = ctx.enter_context(tc.tile_pool(name="aot", bufs=3))
        self.rcp = ctx.enter_context(tc.tile_pool(name="rec", bufs=2))
        # head-0 half of wo, the AllToAll landing tiles and the h0 partial
        # sums are resident from the start so out-proj(h0) never waits on
        # scope-B SBUF to free
        wop0 = ctx.enter_context(tc.tile_pool(name="wo0", bufs=1))
        self.wo_h = [wop0.tile([128, cfg.NHB // cfg.HPC, cfg.D], F16,
                               name="wo_h0"), None]
        self.aip = ctx.enter_context(tc.tile_pool(name="aot_sb", bufs=1))
        self.accp = ctx.enter_context(tc.tile_pool(name="oacc", bufs=1))
        self.aot_sb = {}
        self.oacc = {}
        self.psS = ctx.enter_context(
            tc.tile_pool(name="psS", bufs=2, space="PSUM"))
        self.psO = ctx.enter_context(
            tc.tile_pool(name="psO", bufs=1, space="PSUM"))
        self.psJ = ctx.enter_context(
            tc.tile_pool(name="psJ", bufs=4, space="PSUM"))

    # ---- scope B (projection phase) pools ----
    def open_B(self, ctx):
        tc, cfg = self.tc, self.cfg
        self.xp = ctx.enter_context(tc.tile_pool(name="xslab", bufs=8))
        self.wp = ctx.enter_context(tc.tile_pool(name="wpanel", bufs=2))
        self.tbp = ctx.enter_context(tc.tile_pool(name="tables", bufs=1))
        self.rp = ctx.enter_context(tc.tile_pool(name="ropet", bufs=3))
        self.stp = ctx.enter_context(tc.tile_pool(name="vstage", bufs=2))
        self.psX = ctx.enter_context(
            tc.tile_pool(name="psX", bufs=1, space="PSUM"))
        self.cos_sb = self.tbp.tile([128, cfg.S], F16, name="cos")
        self.sin_sb = self.tbp.tile([128, cfg.S], F16, name="sin")
        self.wq_sb = {}
        self.tables_loaded = False

    def load_tables(self):
        if self.tables_loaded:
            return
        self.tables_loaded = True
        nc = self.nc
        nc.sync.dma_start(out=self.band_sb, in_=self.d["band"][:, :])
        nc.sync.dma_start(out=self.cos_sb, in_=self.d["cosT"][:, :])
        nc.sync.dma_start(out=self.sin_sb, in_=self.d["sinT"][:, :])

    def load_w_one(self, h, proj):
        if (h, proj) in self.wq_sb:
            return self.wq_sb[(h, proj)]
        nc, cfg = self.nc, self.cfg
        dram = (self.d["wq"], self.d["wk"], self.d["wv"])[proj]
        t = self.wp.tile([128, cfg.NDB, 128], F16, tag=f"w{proj}",
                         name=f"w{proj}_{h}")
        # host pre-arranged: [128p, h, db, 128c] contiguous per partition
        src = dram[:, h * cfg.NDB * 128:(h + 1) * cfg.NDB * 128]
        nc.sync.dma_start(out=t, in_=src.rearrange(
            "p (db c) -> p db c", c=128))
        self.wq_sb[(h, proj)] = t
        return t

    def slab(self, seg, b, sj):
        """x^T slab for batch b, slab index sj (SG cols), as a list of
        part-tiles each covering pdb contraction blocks.  The first slab is
        split 4 ways so the first matmuls gate on 512KB, not 2MB.

        Keyed per segment: slabs are re-loaded when a batch comes around
        again for the second head (slots cycle)."""
        nc, cfg = self.nc, self.cfg
        key = (seg, sj)
        if key in self.slab_cache:
            return self.slab_cache[key]
        nparts = 4
        pdb = cfg.NDB // nparts
        off = (b * cfg.S + sj * cfg.SG)
        src = self.d["xT"][:, off:off + cfg.SG].rearrange(
            "(db p) c -> p db c", p=128)
        parts = []
        for pi in range(nparts):
            t = self.xp.tile([128, pdb, cfg.SG], F16, tag="xsp",
                             name=f"xs{seg}_{sj}_{pi}")
            nc.sync.dma_start(out=t, in_=src[:, pi * pdb:(pi + 1) * pdb, :])
            parts.append((t, pdb))
        self.slab_cache[key] = parts
        return parts

    def rope(self, acc, dst, scol):
        """dst[f16] = RoPE(acc[PSUM f32]) using halves layout.

        cos table full 128 rows; sin table sign-folded (rows 0:64 = -sin,
        rows 64:128 = +sin).
        """
        nc, cfg = self.nc, self.cfg
        ST = cfg.ST
        t1 = self.rp.tile([128, ST], F16, tag="t1")
        nc.vector.tensor_mul(t1, acc, self.cos_sb[:, scol:scol + ST])
        t2 = self.rp.tile([128, ST], F16, tag="t2")
        nc.vector.tensor_mul(t2[0:64], acc[64:128],
                             self.sin_sb[0:64, scol:scol + ST])
        nc.vector.tensor_mul(t2[64:128], acc[0:64],
                             self.sin_sb[64:128, scol:scol + ST])
        nc.vector.tensor_add(dst, t1, t2)

    def proj_section(self, seg, b, h, sj, sec, first=False):
        """Generator: project q,k,v for 512 cols (slab sj, section sec).

        first=True sequences the Q chain before the K-weight DMA so the very
        first matmul gates on the minimum number of bytes."""
        nc, cfg = self.nc, self.cfg
        ST, NDB = cfg.ST, cfg.NDB
        wq = self.load_w_one(h, 0)
        parts = self.slab(seg, b, sj)
        if not first:
            wk = self.load_w_one(h, 1)
            wv = self.load_w_one(h, 2)
            self.load_tables()
        scol = sj * cfg.SG + sec * ST            # position within batch b
        gcol = b * cfg.S + scol                  # column in QT/KT
        c0 = sec * ST

        def xsl(db):
            t, pdb = parts[db // parts[0][1]]
            return t[:, db % pdb, c0:c0 + ST]

        accQ = self.psJ.tile([128, ST], F32, tag="pj", name=f"aQ{b}{h}{sj}{sec}")
        accK = self.psJ.tile([128, ST], F32, tag="pj", name=f"aK{b}{h}{sj}{sec}")
        if first:
            for db in range(NDB):
                nc.tensor.matmul(accQ, lhsT=wq[:, db, :], rhs=xsl(db),
                                 start=(db == 0), stop=(db == NDB - 1))
                yield
            wk = self.load_w_one(h, 1)
            wv = self.load_w_one(h, 2)
            self.load_tables()
            for db in range(NDB):
                nc.tensor.matmul(accK, lhsT=wk[:, db, :], rhs=xsl(db),
                                 start=(db == 0), stop=(db == NDB - 1))
                yield
        else:
            for db in range(NDB):
                nc.tensor.matmul(accQ, lhsT=wq[:, db, :], rhs=xsl(db),
                                 start=(db == 0), stop=(db == NDB - 1))
                yield
                nc.tensor.matmul(accK, lhsT=wk[:, db, :], rhs=xsl(db),
                                 start=(db == 0), stop=(db == NDB - 1))
                yield
        self.rope(accQ, self.QT[:, h, gcol:gcol + ST], scol)
        self.rope(accK, self.KT[:, h, gcol:gcol + ST], scol)
        accV = self.psJ.tile([128, ST], F32, tag="pj", name=f"aV{b}{h}{sj}{sec}")
        for db in range(NDB):
            nc.tensor.matmul(accV, lhsT=wv[:, db, :], rhs=xsl(db),
                             start=(db == 0), stop=(db == NDB - 1))
            yield
        vstage = self.stp.tile([128, ST], BF16, tag="vst")
        nc.scalar.copy(out=vstage, in_=accV)
        for jj in range(ST // 128):
            kblk = scol // 128 + jj
            pst = self.psX.tile([128, 128], BF16, tag="px", name="pst")
            nc.tensor.transpose(pst, vstage[:, jj * 128:(jj + 1) * 128],
                                self.ident)
            nc.scalar.copy(out=self.Vhat[:, h, b, kblk, :], in_=pst)
        yield

    def attn_qtile(self, b, h, qt):
        """Generator: attention for q-tile qt of (b,h); yields per k-block."""
        nc, cfg = self.nc, self.cfg
        S, ST = cfg.S, cfg.ST
        q0 = qt * ST
        nkb = (q0 + ST) // 128
        po = self.psO.tile([128, ST], F32, tag="po")
        pa = self.pap.tile([128, ST], BF16, tag="pa")
        for kb in range(nkb):
            d = kb * 128 - q0
            c0 = max(0, d)          # first un-masked q column of this block
            W = ST - c0
            ps = self.psS.tile([128, ST], F32, tag="ps", name=f"ps{kb%2}")
            nc.tensor.matmul(
                ps[:, c0:],
                lhsT=self.KT[:, h, b * S + kb * 128:b * S + kb * 128 + 128],
                rhs=self.QT[:, h, b * S + q0 + c0:b * S + q0 + ST],
                start=True, stop=True)
            pt = self.ptp.tile([128, ST], BF16, tag="pt")
            nc.scalar.activation(pt[:, c0:], ps[:, c0:], EXP)
            if d >= 0:
                nc.vector.tensor_mul(
                    pt[:, c0:], pt[:, c0:],
                    self.band_sb[:, ST - 128:(2 * ST - 128) - d])
            nc.tensor.matmul(po[:, c0:], lhsT=self.Vhat[:, h, b, kb, :],
                             rhs=pt[:, c0:],
                             start=(kb == 0), stop=(kb == nkb - 1))
            if kb == 0:
                nc.vector.tensor_copy(pa, pt)
            elif kb % 2 == 0:
                nc.vector.tensor_add(pa[:, c0:], pa[:, c0:], pt[:, c0:])
            else:
                nc.gpsimd.tensor_add(pa[:, c0:], pa[:, c0:], pt[:, c0:])
            yield
        pd = self.psS.tile([128, ST], F32, tag="ps", name="pd")
        nc.tensor.matmul(pd, lhsT=self.ones_sb, rhs=pa, start=True, stop=True)
        rec = self.rcp.tile([128, ST], F32, tag="rec")
        nc.vector.reciprocal_approx_fast(out=rec, in_=pd)
        aot = self.aotp.tile([128, ST], F16, tag="aot")
        nc.vector.tensor_mul(aot, po, rec)
        r0 = b * S + q0
        slot = r0 // cfg.RPC
        nv = len(self.d["a2a_ins"][h])
        hw = ST // nv
        for v in range(nv):
            nc.sync.dma_start(
                out=self.d["a2a_ins"][h][v][slot, :, :],
                in_=aot[:, v * hw:(v + 1) * hw])
        yield

    def a2a(self, h, v):
        nc, cfg = self.nc, self.cfg
        nc.gpsimd.collective_compute(
            "AllToAll",
            mybir.AluOpType.bypass,
            replica_groups=[list(range(cfg.NCORES))],
            ins=[self.d["a2a_ins"][h][v][:, :, :]],
            outs=[self.d["a2a_outs"][h][v][:, :, :]],
        )

    def load_wo(self, h):
        """wo rows for head h's blocks -> wo_h[h] tile [128, 8, D]."""
        nc, cfg = self.nc, self.cfg
        # blocks i = h, h+HPC, ... : strided view over the i dimension
        wosrc = self.d["wo"][:, :].rearrange(
            "(i hh p) c -> p i hh c", p=128, hh=cfg.HPC)
        eng = nc.sync if h == 0 else nc.scalar
        eng.dma_start(out=self.wo_h[h], in_=wosrc[:, :, h, :])

    # ---- scope C (out-projection) ----
    def open_C(self, ctx):
        tc, cfg = self.tc, self.cfg
        wop1 = ctx.enter_context(tc.tile_pool(name="wo1", bufs=1))
        self.wo_h[1] = wop1.tile([128, cfg.NHB // cfg.HPC, cfg.D], F16,
                                 name="wo_h1")
        self.obp = ctx.enter_context(tc.tile_pool(name="ob", bufs=4))
        self.load_wo(1)

    def load_aot(self, h, v):
        """aot landing tiles for (head, row-part), split by source-core
        half so the first out-proj chain gates on 256KB.  Emitted on the
        sync queue at a point where no later sync DMA is needed sooner
        (HWDGE FIFOs block head-of-line on the collective wait)."""
        nc, cfg = self.nc, self.cfg
        nv = len(self.d["a2a_outs"][h])
        hc = cfg.NCORES // 2
        tiles = []
        for g in range(2):
            a = self.aip.tile([128, hc, cfg.RPC // nv], F16,
                              tag=f"ai{h}{v}{g}", name=f"ai_sb{h}{v}{g}")
            nc.sync.dma_start(
                out=a,
                in_=self.d["a2a_outs"][h][v][g * hc:(g + 1) * hc, :, :]
                .rearrange("c p r -> p c r"))
            tiles.append(a)
        self.aot_sb[(h, v)] = tiles

    def outproj_head(self, h):
        """Generator: head h's contribution to out rows.

        Rounds of (ss, dct-pair) use 2 PSUM banks each so two rounds
        overlap in the 4-slot pj group; yields per contraction block."""
        nc, cfg = self.nc, self.cfg
        D, HPC, RPC, NHB = cfg.D, cfg.HPC, cfg.RPC, cfg.NHB
        NB = NHB // HPC
        wo = self.wo_h[h]
        nv = len(self.d["a2a_outs"][h])
        sspp = (RPC // 128) // nv          # ss strips per row-part
        NDC = D // 512
        hc = self.cfg.NCORES // 2
        for ss in range(RPC // 128):
            aotg, lr = self.aot_sb[(h, ss // sspp)], (ss % sspp) * 128

            def lhs(ib):
                return aotg[ib // hc][:, ib % hc, lr:lr + 128]

            if h == 0:
                # single-bank rounds: fits the spare psJ slot while the
                # last segment's projections still run
                for dct in range(NDC):
                    pu = self.psJ.tile([128, 512], F32, tag="pj",
                                       name=f"pu{dct}")
                    for ib in range(NB):
                        nc.tensor.matmul(
                            pu, lhsT=lhs(ib),
                            rhs=wo[:, ib, dct * 512:(dct + 1) * 512],
                            start=(ib == 0), stop=(ib == NB - 1))
                        yield
                    t = self.accp.tile([128, 512], F16,
                                       tag=f"acc{ss}_{dct}",
                                       name=f"acc{ss}_{dct}")
                    nc.scalar.copy(out=t, in_=pu)
                    self.oacc[(ss, dct)] = t
                    yield
            else:
                for half in range(NDC // 2):
                    pu = [self.psJ.tile([128, 512], F32, tag="pj",
                                        name=f"pu{half}{j}") for j in range(2)]
                    for ib in range(NB):
                        for j in range(2):
                            nc.tensor.matmul(
                                pu[j], lhsT=lhs(ib),
                                rhs=wo[:, ib, (half * 2 + j) * 512:
                                       (half * 2 + j + 1) * 512],
                                start=(ib == 0), stop=(ib == NB - 1))
                        yield
                    ob = self.obp.tile([128, 1024], F32, tag="ob")
                    for j in range(2):
                        nc.vector.tensor_add(
                            ob[:, j * 512:(j + 1) * 512], pu[j],
                            self.oacc[(ss, half * 2 + j)])
                    nc.sync.dma_start(
                        out=self.d["out_rows"][ss * 128:(ss + 1) * 128,
                                               half * 1024:(half + 1) * 1024],
                        in_=ob)
                    yield


def _weave(main_gen, feeders, drain=True):
    """Drive main_gen; after each main step, advance feeder generators so
    their progress fraction tracks the main fraction.  feeders is a list of
    [gen, total_steps, done_steps]."""
    main_steps = 0
    main_total = max(1, main_gen[1])
    gen = main_gen[0]
    while True:
        try:
            next(gen)
        except StopIteration:
            break
        main_steps += 1
        frac = min(1.0, main_steps / main_total)
        for f in feeders:
            target = int(round(f[1] * frac))
            while f[2] < target:
                if next(f[0], StopIteration) is StopIteration:
                    f[2] = f[1]
                    break
                f[2] += 1
    if drain:
        for f in feeders:
            while next(f[0], StopIteration) is not StopIteration:
                f[2] += 1


def build_bass(cfg: Cfg) -> bass.Bass:
    nc = bacc.Bacc("TRN2", target_bir_lowering=False, debug=False,
                   num_devices=cfg.NCORES)
    B, S, D = cfg.B, cfg.S, cfg.D
    HPC, HD, RPC = cfg.HPC, cfg.HD, cfg.RPC

    drams = dict(
        xT=nc.declare_dram_parameter("xT", [D, cfg.BS], F16, isOutput=False),
        wq=nc.declare_dram_parameter("wq", [128, HPC * (D // 128) * HD], F16,
                                     isOutput=False),
        wk=nc.declare_dram_parameter("wk", [128, HPC * (D // 128) * HD], F16,
                                     isOutput=False),
        wv=nc.declare_dram_parameter("wv", [128, HPC * (D // 128) * HD], F16,
                                     isOutput=False),
        wo=nc.declare_dram_parameter("wo", [cfg.NCORES * HPC * HD, D], F16,
                                     isOutput=False),
        cosT=nc.declare_dram_parameter("cosT", [128, S], F16, isOutput=False),
        sinT=nc.declare_dram_parameter("sinT", [128, S], F16, isOutput=False),
        band=nc.declare_dram_parameter("band", [128, 2 * cfg.ST - 128], BF16,
                                       isOutput=False),
        out_rows=nc.declare_dram_parameter("out_rows", [RPC, D], F32,
                                           isOutput=True),
        # head 0's AllToAll is fully overlapped -> one op (a second gpsimd
        # trigger would head-block the pa-adds on the GpSimd queue).  Head
        # 1's runs at the tail -> split in row-halves to pipeline with
        # out-projection (a 512KB collective also measures far cheaper).
        a2a_ins=[[nc.dram_tensor(f"a2a_in{h}{v}",
                                 [cfg.NCORES, HD, RPC // (1 + h)], F16)
                  for v in range(1 + h)] for h in range(HPC)],
        a2a_outs=[[nc.dram_tensor(f"a2a_out{h}{v}",
                                  [cfg.NCORES, HD, RPC // (1 + h)], F16)
                   for v in range(1 + h)] for h in range(HPC)],
    )

    # segment order: h-major (b fast) so head-0's AllToAll fires at 50%
    segs = [(0, 0), (1, 0), (0, 1), (1, 1)]

    with tile.TileContext(nc) as tc:
        with ExitStack() as ctxA:
            em = _Emitter(nc, tc, cfg, drams)
            em.open_A(ctxA)

            # proj units: (seg_idx, slab_j, section) — 512 cols each;
            # attn units: (seg_idx, qt)
            punits = [(k, sj, sec) for k in range(4)
                      for sj in range(S // cfg.SG) for sec in range(cfg.SPS)]
            aunits = [(k, qt) for k in range(4) for qt in range(cfg.NQT)]
            # pairing: A(k,qt) ⊗ P(unit qt+2 later)  (prologue: first 2 P)
            PW = 2 * cfg.NDB + cfg.NDB + 1     # yields per proj section

            def AW(qt):
                return (qt + 1) * (cfg.ST // 128) + 1

            with ExitStack() as ctxB:
                em.open_B(ctxB)
                # prologue: first proj section straight, minimal DMA gating
                for (k, sj, sec) in punits[:1]:
                    b, h = segs[k]
                    for _ in em.proj_section(k, b, h, sj, sec, first=True):
                        pass
                # main weave at lag 1; out-proj(h0) weaves into the whole
                # last segment's attention (its AllToAll landed at ~55%)
                # main weave at lag 1; the last attention q-tile runs in
                # scope C woven with the start of out-proj head 0
                for ai, (k, qt) in enumerate(aunits[:-1]):
                    b, h = segs[k]
                    feeders = []
                    pi = ai + 1
                    if pi < len(punits):
                        pk, psj, psec = punits[pi]
                        pb, ph = segs[pk]
                        feeders.append(
                            [em.proj_section(pk, pb, ph, psj, psec), PW, 0])
                    _weave((em.attn_qtile(b, h, qt), AW(qt)), feeders)
                    if k == 1 and qt == cfg.NQT - 1:
                        em.a2a(0, 0)
                        em.load_wo(0)
                # h0 landing tile: emitted after the last slab DMA so the
                # collective-gated wait can't head-block the slab stream
                em.load_aot(0, 0)

            with ExitStack() as ctxC:
                em.open_C(ctxC)
                k, qt = aunits[-1]
                b, h = segs[k]
                for _ in em.attn_qtile(b, h, qt):
                    pass
                em.a2a(1, 0)
                em.a2a(1, 1)
                # all of out-proj(h0) held to here: it is the only PE work
                # that can cover the tail AllToAll's (variable) latency
                for _ in em.outproj_head(0):
                    pass
                em.load_aot(1, 0)
                em.load_aot(1, 1)
                for _ in em.outproj_head(1):
                    pass

    nc.finalize()
    return nc


# ---------------------------------------------------------------------------
# Host side
# ---------------------------------------------------------------------------

def _rope_perm(hd):
    return np.concatenate([np.arange(0, hd, 2), np.arange(1, hd, 2)])


def prepare_inputs(cfg: Cfg, x, freq_cis, wq_base, wk_base, wv_base, head_a,
                   head_b, q_a, q_b, k_a, k_b, v_a, v_b, wo):
    """Fold LoRA + softmax scale + RoPE permutation into per-core weights."""
    B, S, D, HD, HPC, NC_ = cfg.B, cfg.S, cfg.D, cfg.HD, cfg.HPC, cfg.NCORES
    HEADS = HPC * NC_
    LORA_SCALE = 2.0
    sm = 1.0 / math.sqrt(HD)

    def fold(w_base, oa, ob):
        w = w_base.astype(np.float64).copy()
        only = LORA_SCALE * (oa.astype(np.float64) @ ob.astype(np.float64))
        hoff = LORA_SCALE * (head_a.astype(np.float64)
                             @ head_b.astype(np.float64))
        w += hoff
        w += np.tile(only, (1, HEADS))
        return w

    wq_eff = fold(wq_base, q_a, q_b) * sm
    wk_eff = fold(wk_base, k_a, k_b)
    wv_eff = fold(wv_base, v_a, v_b)

    perm = _rope_perm(HD)
    for h in range(HEADS):
        cols = h * HD + perm
        wq_eff[:, h * HD:(h + 1) * HD] = wq_eff[:, cols]
        wk_eff[:, h * HD:(h + 1) * HD] = wk_eff[:, cols]
    wq_eff = wq_eff.astype(np.float16)
    wk_eff = wk_eff.astype(np.float16)
    wv_eff = wv_eff.astype(np.float16)

    xT = np.ascontiguousarray(x.reshape(cfg.BS, D).T.astype(np.float16))

    cos = freq_cis[:S, :, 0].T.astype(np.float32)   # [64, S]
    sin = freq_cis[:S, :, 1].T.astype(np.float32)
    cosT = np.ascontiguousarray(
        np.concatenate([cos, cos], axis=0)).astype(np.float16)
    # sign-folded: top half -sin (for out_lo = lo*cos - hi*sin),
    # bottom half +sin (for out_hi = hi*cos + lo*sin)
    sinT = np.ascontiguousarray(
        np.concatenate([-sin, sin], axis=0)).astype(np.float16)

    ST = cfg.ST
    ii = np.arange(128)[:, None]
    cc = np.arange(2 * ST - 128)[None, :]
    band01 = np.where(ii <= cc - (ST - 128), 1.0, 0.0).astype(
        ml_dtypes.bfloat16)

    def wpanels(w, c):
        """[D, HPC*HD] core slice -> [128, HPC*NDB*128]: per head, per db
        block, contiguous 128 cols per partition."""
        NDB = D // 128
        sl = w[:, c * HPC * HD:(c + 1) * HPC * HD]         # [D, HPC*128]
        a = sl.reshape(NDB, 128, HPC, HD)                   # [db, p, h, c]
        a = a.transpose(1, 2, 0, 3).reshape(128, HPC * NDB * HD)
        return np.ascontiguousarray(a)

    in_maps = []
    for c in range(NC_):
        in_maps.append(dict(
            xT=xT,
            wq=wpanels(wq_eff, c),
            wk=wpanels(wk_eff, c),
            wv=wpanels(wv_eff, c),
            wo=np.ascontiguousarray(wo.astype(np.float16)),
            cosT=cosT, sinT=sinT, band=band01,
        ))
    return in_maps


_BASS_CACHE = {}


def _get_bass(cfg: Cfg):
    key = (cfg.B, cfg.S, cfg.D, cfg.HPC, cfg.NCORES)
    if key not in _BASS_CACHE:
        _BASS_CACHE[key] = build_bass(cfg)
    return _BASS_CACHE[key]


def kernel(**inputs) -> np.ndarray:
    from concourse.bass_utils import run_bass_kernel_spmd

    x = np.asarray(inputs["x"])
    B, S, D = x.shape
    cfg = Cfg(B=B, S=S, D=D, HEADS=16, NCORES=8)
    in_maps = prepare_inputs(cfg, **{k: np.asarray(v)
                                     for k, v in inputs.items()})
    nc = _get_bass(cfg)
    res = run_bass_kernel_spmd(nc, in_maps, list(range(cfg.NCORES)))
    rows = np.concatenate([res.results[c]["out_rows"]
                           for c in range(cfg.NCORES)], axis=0)
    return rows.reshape(B, S, D).astype(np.float32)


# revision 71
# speedup vs baseline: 1.0175x; 1.0175x over previous
"""Trainium2 Bass kernel for LoRA-augmented causal attention.

Reference computation (per nn_Attention_31688268710508):
  x:(B,S,D) -> q/k/v = x@W* + broadcast LoRA + shared head-offset LoRA,
  RoPE(q,k), causal softmax attention per (b,head), out-proj with wo.

Strategy (8 NeuronCores, tensor-parallel over heads):
  * All rank-8 LoRA terms folded into effective projection weights on the
    host; softmax 1/sqrt(HD) folded into Wq; RoPE pair permutation folded
    into Wq/Wk columns so RoPE is a half-partition rotation; W panels
    pre-transposed on the host so their DMAs are contiguous.
  * fp16 operands on the PE (fp32 PSUM); P/V in bf16; the causal mask is
    a 0/1 multiply after exp (bf16 has range for unmasked exp), and the
    fully-masked leading columns of diagonal blocks are trimmed from the
    scores/PV matmuls.
  * Fine-grained weave: proj 512-col sections interleave with attention
    q-tile steps at one-section lag so the PE never waits on exp/RoPE
    latency; 4 PSUM slots decouple proj matmuls from RoPE evacuation;
    softmax denominators accumulate via DVE/GpSimd adds + one ones-matmul.
  * h-major segment order: head 0's AllToAll fires at the 50% mark fully
    hidden; head 1's runs at the tail split into row-halves (small
    collectives are several times cheaper), covered by head 0's
    out-projection, with head 1's out-projection gated per row-half.
  * HWDGE queues are FIFO and block head-of-line on gated DMAs, so
    collective-gated loads are emitted only at points where nothing
    later on that queue is needed sooner.
"""

import math
import os
import sys
from contextlib import ExitStack

import ml_dtypes
import numpy as np

for _p in ("/opt/trn_rl_repo", "/root/.axon_site/_ro/trn_rl_repo"):
    if os.path.isdir(_p) and _p not in sys.path:
        sys.path.insert(0, _p)

import concourse.bass as bass  # noqa: E402
import concourse.mybir as mybir  # noqa: E402
import concourse.tile as tile  # noqa: E402
from concourse import bacc  # noqa: E402
from concourse.masks import make_identity  # noqa: E402

F32 = mybir.dt.float32
F16 = mybir.dt.float16
BF16 = mybir.dt.bfloat16
EXP = mybir.ActivationFunctionType.Exp


class Cfg:
    def __init__(self, B=2, S=2048, D=2048, HEADS=16, NCORES=8):
        self.B, self.S, self.D, self.NCORES = B, S, D, NCORES
        self.HD = 128
        self.HPC = HEADS // NCORES          # heads per core
        self.BS = B * S
        self.RPC = self.BS // NCORES        # output rows per core
        self.ST = 512                       # q tile / proj col tile
        self.SG = 512                       # x^T slab width
        self.NDB = D // 128                 # contraction blocks
        self.NHB = (HEADS * self.HD) // 128  # out-proj contraction blocks
        self.NQT = S // self.ST             # q tiles per (b,h)
        self.SPS = self.SG // self.ST       # 512-sections per slab
        assert self.HD == 128 and S % self.ST == 0 and self.RPC == self.ST


class _Emitter:
    """Holds all tiles/pools and emits proj/attention/outproj units."""

    def __init__(self, nc, tc, cfg, drams):
        self.nc, self.tc, self.cfg = nc, tc, cfg
        self.d = drams
        self.slab_cache = {}

    # ---- scope A (whole kernel) pools ----
    def open_A(self, ctx):
        tc, cfg = self.tc, self.cfg
        constp = ctx.enter_context(tc.tile_pool(name="const", bufs=1))
        self.ident = constp.tile([128, 128], BF16)
        make_identity(self.nc, self.ident)
        self.ones_sb = constp.tile([128, 128], BF16, name="ones")
        self.nc.vector.memset(self.ones_sb, 1.0)
        self.band_sb = constp.tile([128, 2 * cfg.ST - 128], BF16, name="band")

        qkv = ctx.enter_context(tc.tile_pool(name="qkv", bufs=1))
        self.QT = qkv.tile([128, cfg.HPC, cfg.BS], F16, name="QT")
        self.KT = qkv.tile([128, cfg.HPC, cfg.BS], F16, name="KT")
        self.Vhat = qkv.tile([128, cfg.HPC, cfg.B, cfg.S // 128, 128], BF16,
                             name="Vhat")

        self.ptp = ctx.enter_context(tc.tile_pool(name="ptile", bufs=4))
        self.pap = ctx.enter_context(tc.tile_pool(name="pacc", bufs=2))
        self.aotp = ctx.enter_context(tc.tile_pool(name="aot", bufs=3))
        self.rcp = ctx.enter_context(tc.tile_pool(name="rec", bufs=2))
        # head-0 half of wo, the AllToAll landing tiles and the h0 partial
        # sums are resident from the start so out-proj(h0) never waits on
        # scope-B SBUF to free
        wop0 = ctx.enter_context(tc.tile_pool(name="wo0", bufs=1))
        self.wo_h = [wop0.tile([128, cfg.NHB // cfg.HPC, cfg.D], F16,
                               name="wo_h0"), None]
        self.aip = ctx.enter_context(tc.tile_pool(name="aot_sb", bufs=1))
        self.accp = ctx.enter_context(tc.tile_pool(name="oacc", bufs=1))
        self.aot_sb = {}
        self.oacc = {}
        self.psS = ctx.enter_context(
            tc.tile_pool(name="psS", bufs=2, space="PSUM"))
        self.psO = ctx.enter_context(
            tc.tile_pool(name="psO", bufs=1, space="PSUM"))
        self.psJ = ctx.enter_context(
            tc.tile_pool(name="psJ", bufs=4, space="PSUM"))

    # ---- scope B (projection phase) pools ----
    def open_B(self, ctx):
        tc, cfg = self.tc, self.cfg
        self.xp = ctx.enter_context(tc.tile_pool(name="xslab", bufs=8))
        self.wp = ctx.enter_context(tc.tile_pool(name="wpanel", bufs=2))
        self.tbp = ctx.enter_context(tc.tile_pool(name="tables", bufs=1))
        self.rp = ctx.enter_context(tc.tile_pool(name="ropet", bufs=3))
        self.stp = ctx.enter_context(tc.tile_pool(name="vstage", bufs=2))
        self.psX = ctx.enter_context(
            tc.tile_pool(name="psX", bufs=1, space="PSUM"))
        self.cos_sb = self.tbp.tile([128, cfg.S], F16, name="cos")
        self.sin_sb = self.tbp.tile([128, cfg.S], F16, name="sin")
        self.wq_sb = {}
        self.tables_loaded = False

    def load_tables(self):
        if self.tables_loaded:
            return
        self.tables_loaded = True
        nc = self.nc
        nc.sync.dma_start(out=self.band_sb, in_=self.d["band"][:, :])
        nc.sync.dma_start(out=self.cos_sb, in_=self.d["cosT"][:, :])
        nc.sync.dma_start(out=self.sin_sb, in_=self.d["sinT"][:, :])

    def load_w_one(self, h, proj):
        if (h, proj) in self.wq_sb:
            return self.wq_sb[(h, proj)]
        nc, cfg = self.nc, self.cfg
        dram = (self.d["wq"], self.d["wk"], self.d["wv"])[proj]
        t = self.wp.tile([128, cfg.NDB, 128], F16, tag=f"w{proj}",
                         name=f"w{proj}_{h}")
        # host pre-arranged: [128p, h, db, 128c] contiguous per partition
        src = dram[:, h * cfg.NDB * 128:(h + 1) * cfg.NDB * 128]
        nc.sync.dma_start(out=t, in_=src.rearrange(
            "p (db c) -> p db c", c=128))
        self.wq_sb[(h, proj)] = t
        return t

    def slab(self, seg, b, sj):
        """x^T slab for batch b, slab index sj (SG cols), as a list of
        part-tiles each covering pdb contraction blocks.  The first slab is
        split 4 ways so the first matmuls gate on 512KB, not 2MB.

        Keyed per segment: slabs are re-loaded when a batch comes around
        again for the second head (slots cycle)."""
        nc, cfg = self.nc, self.cfg
        key = (seg, sj)
        if key in self.slab_cache:
            return self.slab_cache[key]
        nparts = 4
        pdb = cfg.NDB // nparts
        off = (b * cfg.S + sj * cfg.SG)
        src = self.d["xT"][:, off:off + cfg.SG].rearrange(
            "(db p) c -> p db c", p=128)
        parts = []
        for pi in range(nparts):
            t = self.xp.tile([128, pdb, cfg.SG], F16, tag="xsp",
                             name=f"xs{seg}_{sj}_{pi}")
            nc.sync.dma_start(out=t, in_=src[:, pi * pdb:(pi + 1) * pdb, :])
            parts.append((t, pdb))
        self.slab_cache[key] = parts
        return parts

    def rope(self, acc, dst, scol):
        """dst[f16] = RoPE(acc[PSUM f32]) using halves layout.

        cos table full 128 rows; sin table sign-folded (rows 0:64 = -sin,
        rows 64:128 = +sin).
        """
        nc, cfg = self.nc, self.cfg
        ST = cfg.ST
        t1 = self.rp.tile([128, ST], F16, tag="t1")
        nc.vector.tensor_mul(t1, acc, self.cos_sb[:, scol:scol + ST])
        t2 = self.rp.tile([128, ST], F16, tag="t2")
        nc.vector.tensor_mul(t2[0:64], acc[64:128],
                             self.sin_sb[0:64, scol:scol + ST])
        nc.vector.tensor_mul(t2[64:128], acc[0:64],
                             self.sin_sb[64:128, scol:scol + ST])
        nc.vector.tensor_add(dst, t1, t2)

    def proj_section(self, seg, b, h, sj, sec, first=False):
        """Generator: project q,k,v for 512 cols (slab sj, section sec).

        first=True sequences the Q chain before the K-weight DMA so the very
        first matmul gates on the minimum number of bytes."""
        nc, cfg = self.nc, self.cfg
        ST, NDB = cfg.ST, cfg.NDB
        if first:
            # split q-weight panel: the very first matmuls gate on 256KB
            wqa = self.wp.tile([128, NDB // 2, 128], F16, tag="w0a",
                               name="wq_first_a")
            wqb = self.wp.tile([128, NDB // 2, 128], F16, tag="w0b",
                               name="wq_first_b")
            src = self.d["wq"][:, h * NDB * 128:(h + 1) * NDB * 128]
            srcr = src.rearrange("p (db c) -> p db c", c=128)
            nc.sync.dma_start(out=wqa, in_=srcr[:, :NDB // 2, :])
        else:
            wq = self.load_w_one(h, 0)
        parts = self.slab(seg, b, sj)
        if first:
            nc.sync.dma_start(out=wqb, in_=srcr[:, NDB // 2:, :])
        else:
            wk = self.load_w_one(h, 1)
            wv = self.load_w_one(h, 2)
            self.load_tables()
        scol = sj * cfg.SG + sec * ST            # position within batch b
        gcol = b * cfg.S + scol                  # column in QT/KT
        c0 = sec * ST

        def xsl(db):
            t, pdb = parts[db // parts[0][1]]
            return t[:, db % pdb, c0:c0 + ST]

        accQ = self.psJ.tile([128, ST], F32, tag="pj", name=f"aQ{b}{h}{sj}{sec}")
        accK = self.psJ.tile([128, ST], F32, tag="pj", name=f"aK{b}{h}{sj}{sec}")
        if first:
            for db in range(NDB):
                wt = wqa if db < NDB // 2 else wqb
                nc.tensor.matmul(accQ, lhsT=wt[:, db % (NDB // 2), :],
                                 rhs=xsl(db),
                                 start=(db == 0), stop=(db == NDB - 1))
                yield
            wk = self.load_w_one(h, 1)
            wv = self.load_w_one(h, 2)
            self.load_w_one(h, 0)      # full panel for later sections
            self.load_tables()
            for db in range(NDB):
                nc.tensor.matmul(accK, lhsT=wk[:, db, :], rhs=xsl(db),
                                 start=(db == 0), stop=(db == NDB - 1))
                yield
        else:
            for db in range(NDB):
                nc.tensor.matmul(accQ, lhsT=wq[:, db, :], rhs=xsl(db),
                                 start=(db == 0), stop=(db == NDB - 1))
                yield
                nc.tensor.matmul(accK, lhsT=wk[:, db, :], rhs=xsl(db),
                                 start=(db == 0), stop=(db == NDB - 1))
                yield
        self.rope(accQ, self.QT[:, h, gcol:gcol + ST], scol)
        self.rope(accK, self.KT[:, h, gcol:gcol + ST], scol)
        accV = self.psJ.tile([128, ST], F32, tag="pj", name=f"aV{b}{h}{sj}{sec}")
        for db in range(NDB):
            nc.tensor.matmul(accV, lhsT=wv[:, db, :], rhs=xsl(db),
                             start=(db == 0), stop=(db == NDB - 1))
            yield
        vstage = self.stp.tile([128, ST], BF16, tag="vst")
        nc.scalar.copy(out=vstage, in_=accV)
        for jj in range(ST // 128):
            kblk = scol // 128 + jj
            pst = self.psX.tile([128, 128], BF16, tag="px", name="pst")
            nc.tensor.transpose(pst, vstage[:, jj * 128:(jj + 1) * 128],
                                self.ident)
            nc.scalar.copy(out=self.Vhat[:, h, b, kblk, :], in_=pst)
        yield

    def attn_qtile(self, b, h, qt):
        """Generator: attention for q-tile qt of (b,h); yields per k-block."""
        nc, cfg = self.nc, self.cfg
        S, ST = cfg.S, cfg.ST
        q0 = qt * ST
        nkb = (q0 + ST) // 128
        po = self.psO.tile([128, ST], F32, tag="po")
        pa = self.pap.tile([128, ST], BF16, tag="pa")
        for kb in range(nkb):
            d = kb * 128 - q0
            c0 = max(0, d)          # first un-masked q column of this block
            W = ST - c0
            ps = self.psS.tile([128, ST], F32, tag="ps", name=f"ps{kb%2}")
            nc.tensor.matmul(
                ps[:, c0:],
                lhsT=self.KT[:, h, b * S + kb * 128:b * S + kb * 128 + 128],
                rhs=self.QT[:, h, b * S + q0 + c0:b * S + q0 + ST],
                start=True, stop=True)
            pt = self.ptp.tile([128, ST], BF16, tag="pt")
            nc.scalar.activation(pt[:, c0:], ps[:, c0:], EXP)
            if d >= 0:
                nc.vector.tensor_mul(
                    pt[:, c0:], pt[:, c0:],
                    self.band_sb[:, ST - 128:(2 * ST - 128) - d])
            nc.tensor.matmul(po[:, c0:], lhsT=self.Vhat[:, h, b, kb, :],
                             rhs=pt[:, c0:],
                             start=(kb == 0), stop=(kb == nkb - 1))
            if kb == 0:
                nc.vector.tensor_copy(pa, pt)
            elif kb % 2 == 0:
                nc.vector.tensor_add(pa[:, c0:], pa[:, c0:], pt[:, c0:])
            else:
                nc.gpsimd.tensor_add(pa[:, c0:], pa[:, c0:], pt[:, c0:])
            yield
        pd = self.psS.tile([128, ST], F32, tag="ps", name="pd")
        nc.tensor.matmul(pd, lhsT=self.ones_sb, rhs=pa, start=True, stop=True)
        rec = self.rcp.tile([128, ST], F32, tag="rec")
        nc.vector.reciprocal_approx_fast(out=rec, in_=pd)
        aot = self.aotp.tile([128, ST], F16, tag="aot")
        nc.vector.tensor_mul(aot, po, rec)
        r0 = b * S + q0
        slot = r0 // cfg.RPC
        nv = len(self.d["a2a_ins"][h])
        hw = ST // nv
        for v in range(nv):
            nc.sync.dma_start(
                out=self.d["a2a_ins"][h][v][slot, :, :],
                in_=aot[:, v * hw:(v + 1) * hw])
        yield

    def a2a(self, h, v):
        nc, cfg = self.nc, self.cfg
        nc.gpsimd.collective_compute(
            "AllToAll",
            mybir.AluOpType.bypass,
            replica_groups=[list(range(cfg.NCORES))],
            ins=[self.d["a2a_ins"][h][v][:, :, :]],
            outs=[self.d["a2a_outs"][h][v][:, :, :]],
        )

    def load_wo(self, h):
        """wo rows for head h's blocks -> wo_h[h] tile [128, 8, D]."""
        nc, cfg = self.nc, self.cfg
        # blocks i = h, h+HPC, ... : strided view over the i dimension
        wosrc = self.d["wo"][:, :].rearrange(
            "(i hh p) c -> p i hh c", p=128, hh=cfg.HPC)
        eng = nc.sync if h == 0 else nc.scalar
        eng.dma_start(out=self.wo_h[h], in_=wosrc[:, :, h, :])

    # ---- scope C (out-projection) ----
    def open_C(self, ctx):
        tc, cfg = self.tc, self.cfg
        wop1 = ctx.enter_context(tc.tile_pool(name="wo1", bufs=1))
        self.wo_h[1] = wop1.tile([128, cfg.NHB // cfg.HPC, cfg.D], F16,
                                 name="wo_h1")
        self.obp = ctx.enter_context(tc.tile_pool(name="ob", bufs=4))
        self.load_wo(1)

    def load_aot(self, h, v):
        """aot landing tiles for (head, row-part), split by source-core
        half so the first out-proj chain gates on 256KB.  Emitted on the
        sync queue at a point where no later sync DMA is needed sooner
        (HWDGE FIFOs block head-of-line on the collective wait)."""
        nc, cfg = self.nc, self.cfg
        nv = len(self.d["a2a_outs"][h])
        hc = cfg.NCORES // 2
        tiles = []
        for g in range(2):
            a = self.aip.tile([128, hc, cfg.RPC // nv], F16,
                              tag=f"ai{h}{v}{g}", name=f"ai_sb{h}{v}{g}")
            nc.sync.dma_start(
                out=a,
                in_=self.d["a2a_outs"][h][v][g * hc:(g + 1) * hc, :, :]
                .rearrange("c p r -> p c r"))
            tiles.append(a)
        self.aot_sb[(h, v)] = tiles

    def outproj_head(self, h):
        """Generator: head h's contribution to out rows.

        Rounds of (ss, dct-pair) use 2 PSUM banks each so two rounds
        overlap in the 4-slot pj group; yields per contraction block."""
        nc, cfg = self.nc, self.cfg
        D, HPC, RPC, NHB = cfg.D, cfg.HPC, cfg.RPC, cfg.NHB
        NB = NHB // HPC
        wo = self.wo_h[h]
        nv = len(self.d["a2a_outs"][h])
        sspp = (RPC // 128) // nv          # ss strips per row-part
        NDC = D // 512
        hc = self.cfg.NCORES // 2
        for ss in range(RPC // 128):
            aotg, lr = self.aot_sb[(h, ss // sspp)], (ss % sspp) * 128

            def lhs(ib):
                return aotg[ib // hc][:, ib % hc, lr:lr + 128]

            if h == 0:
                # single-bank rounds: fits the spare psJ slot while the
                # last segment's projections still run
                for dct in range(NDC):
                    pu = self.psJ.tile([128, 512], F32, tag="pj",
                                       name=f"pu{dct}")
                    for ib in range(NB):
                        nc.tensor.matmul(
                            pu, lhsT=lhs(ib),
                            rhs=wo[:, ib, dct * 512:(dct + 1) * 512],
                            start=(ib == 0), stop=(ib == NB - 1))
                        yield
                    t = self.accp.tile([128, 512], F16,
                                       tag=f"acc{ss}_{dct}",
                                       name=f"acc{ss}_{dct}")
                    nc.scalar.copy(out=t, in_=pu)
                    self.oacc[(ss, dct)] = t
                    yield
            else:
                for half in range(NDC // 2):
                    pu = [self.psJ.tile([128, 512], F32, tag="pj",
                                        name=f"pu{half}{j}") for j in range(2)]
                    for ib in range(NB):
                        for j in range(2):
                            nc.tensor.matmul(
                                pu[j], lhsT=lhs(ib),
                                rhs=wo[:, ib, (half * 2 + j) * 512:
                                       (half * 2 + j + 1) * 512],
                                start=(ib == 0), stop=(ib == NB - 1))
                        yield
                    ob = self.obp.tile([128, 1024], F32, tag="ob")
                    for j in range(2):
                        nc.vector.tensor_add(
                            ob[:, j * 512:(j + 1) * 512], pu[j],
                            self.oacc[(ss, half * 2 + j)])
                    nc.sync.dma_start(
                        out=self.d["out_rows"][ss * 128:(ss + 1) * 128,
                                               half * 1024:(half + 1) * 1024],
                        in_=ob)
                    yield


def _weave(main_gen, feeders, drain=True):
    """Drive main_gen; after each main step, advance feeder generators so
    their progress fraction tracks the main fraction.  feeders is a list of
    [gen, total_steps, done_steps]."""
    main_steps = 0
    main_total = max(1, main_gen[1])
    gen = main_gen[0]
    while True:
        try:
            next(gen)
        except StopIteration:
            break
        main_steps += 1
        frac = min(1.0, main_steps / main_total)
        for f in feeders:
            target = int(round(f[1] * frac))
            while f[2] < target:
                if next(f[0], StopIteration) is StopIteration:
                    f[2] = f[1]
                    break
                f[2] += 1
    if drain:
        for f in feeders:
            while next(f[0], StopIteration) is not StopIteration:
                f[2] += 1


def build_bass(cfg: Cfg) -> bass.Bass:
    nc = bacc.Bacc("TRN2", target_bir_lowering=False, debug=False,
                   num_devices=cfg.NCORES)
    B, S, D = cfg.B, cfg.S, cfg.D
    HPC, HD, RPC = cfg.HPC, cfg.HD, cfg.RPC

    drams = dict(
        xT=nc.declare_dram_parameter("xT", [D, cfg.BS], F16, isOutput=False),
        wq=nc.declare_dram_parameter("wq", [128, HPC * (D // 128) * HD], F16,
                                     isOutput=False),
        wk=nc.declare_dram_parameter("wk", [128, HPC * (D // 128) * HD], F16,
                                     isOutput=False),
        wv=nc.declare_dram_parameter("wv", [128, HPC * (D // 128) * HD], F16,
                                     isOutput=False),
        wo=nc.declare_dram_parameter("wo", [cfg.NCORES * HPC * HD, D], F16,
                                     isOutput=False),
        cosT=nc.declare_dram_parameter("cosT", [128, S], F16, isOutput=False),
        sinT=nc.declare_dram_parameter("sinT", [128, S], F16, isOutput=False),
        band=nc.declare_dram_parameter("band", [128, 2 * cfg.ST - 128], BF16,
                                       isOutput=False),
        out_rows=nc.declare_dram_parameter("out_rows", [RPC, D], F32,
                                           isOutput=True),
        # head 0's AllToAll is fully overlapped -> one op (a second gpsimd
        # trigger would head-block the pa-adds on the GpSimd queue).  Head
        # 1's runs at the tail -> split in row-halves to pipeline with
        # out-projection (a 512KB collective also measures far cheaper).
        a2a_ins=[[nc.dram_tensor(f"a2a_in{h}{v}",
                                 [cfg.NCORES, HD, RPC // (1 + h)], F16)
                  for v in range(1 + h)] for h in range(HPC)],
        a2a_outs=[[nc.dram_tensor(f"a2a_out{h}{v}",
                                  [cfg.NCORES, HD, RPC // (1 + h)], F16)
                   for v in range(1 + h)] for h in range(HPC)],
    )

    # segment order: h-major (b fast) so head-0's AllToAll fires at 50%
    segs = [(0, 0), (1, 0), (0, 1), (1, 1)]

    with tile.TileContext(nc) as tc:
        with ExitStack() as ctxA:
            em = _Emitter(nc, tc, cfg, drams)
            em.open_A(ctxA)

            # proj units: (seg_idx, slab_j, section) — 512 cols each;
            # attn units: (seg_idx, qt)
            punits = [(k, sj, sec) for k in range(4)
                      for sj in range(S // cfg.SG) for sec in range(cfg.SPS)]
            aunits = [(k, qt) for k in range(4) for qt in range(cfg.NQT)]
            # pairing: A(k,qt) ⊗ P(unit qt+2 later)  (prologue: first 2 P)
            PW = 2 * cfg.NDB + cfg.NDB + 1     # yields per proj section

            def AW(qt):
                return (qt + 1) * (cfg.ST // 128) + 1

            with ExitStack() as ctxB:
                em.open_B(ctxB)
                # prologue: first proj section straight, minimal DMA gating
                for (k, sj, sec) in punits[:1]:
                    b, h = segs[k]
                    for _ in em.proj_section(k, b, h, sj, sec, first=True):
                        pass
                # main weave at lag 1; out-proj(h0) weaves into the whole
                # last segment's attention (its AllToAll landed at ~55%)
                # main weave at lag 1; the last attention q-tile runs in
                # scope C woven with the start of out-proj head 0
                for ai, (k, qt) in enumerate(aunits[:-1]):
                    b, h = segs[k]
                    feeders = []
                    pi = ai + 1
                    if pi < len(punits):
                        pk, psj, psec = punits[pi]
                        pb, ph = segs[pk]
                        feeders.append(
                            [em.proj_section(pk, pb, ph, psj, psec), PW, 0])
                    _weave((em.attn_qtile(b, h, qt), AW(qt)), feeders)
                    if k == 1 and qt == cfg.NQT - 1:
                        em.a2a(0, 0)
                        em.load_wo(0)
                # h0 landing tile: emitted after the last slab DMA so the
                # collective-gated wait can't head-block the slab stream
                em.load_aot(0, 0)

            with ExitStack() as ctxC:
                em.open_C(ctxC)
                k, qt = aunits[-1]
                b, h = segs[k]
                for _ in em.attn_qtile(b, h, qt):
                    pass
                em.a2a(1, 0)
                em.a2a(1, 1)
                # all of out-proj(h0) held to here: it is the only PE work
                # that can cover the tail AllToAll's (variable) latency
                for _ in em.outproj_head(0):
                    pass
                em.load_aot(1, 0)
                em.load_aot(1, 1)
                for _ in em.outproj_head(1):
                    pass

    nc.finalize()
    return nc


# ---------------------------------------------------------------------------
# Host side
# ---------------------------------------------------------------------------

def _rope_perm(hd):
    return np.concatenate([np.arange(0, hd, 2), np.arange(1, hd, 2)])


def prepare_inputs(cfg: Cfg, x, freq_cis, wq_base, wk_base, wv_base, head_a,
                   head_b, q_a, q_b, k_a, k_b, v_a, v_b, wo):
    """Fold LoRA + softmax scale + RoPE permutation into per-core weights."""
    B, S, D, HD, HPC, NC_ = cfg.B, cfg.S, cfg.D, cfg.HD, cfg.HPC, cfg.NCORES
    HEADS = HPC * NC_
    LORA_SCALE = 2.0
    sm = 1.0 / math.sqrt(HD)

    def fold(w_base, oa, ob):
        w = w_base.astype(np.float64).copy()
        only = LORA_SCALE * (oa.astype(np.float64) @ ob.astype(np.float64))
        hoff = LORA_SCALE * (head_a.astype(np.float64)
                             @ head_b.astype(np.float64))
        w += hoff
        w += np.tile(only, (1, HEADS))
        return w

    wq_eff = fold(wq_base, q_a, q_b) * sm
    wk_eff = fold(wk_base, k_a, k_b)
    wv_eff = fold(wv_base, v_a, v_b)

    perm = _rope_perm(HD)
    for h in range(HEADS):
        cols = h * HD + perm
        wq_eff[:, h * HD:(h + 1) * HD] = wq_eff[:, cols]
        wk_eff[:, h * HD:(h + 1) * HD] = wk_eff[:, cols]
    wq_eff = wq_eff.astype(np.float16)
    wk_eff = wk_eff.astype(np.float16)
    wv_eff = wv_eff.astype(np.float16)

    xT = np.ascontiguousarray(x.reshape(cfg.BS, D).T.astype(np.float16))

    cos = freq_cis[:S, :, 0].T.astype(np.float32)   # [64, S]
    sin = freq_cis[:S, :, 1].T.astype(np.float32)
    cosT = np.ascontiguousarray(
        np.concatenate([cos, cos], axis=0)).astype(np.float16)
    # sign-folded: top half -sin (for out_lo = lo*cos - hi*sin),
    # bottom half +sin (for out_hi = hi*cos + lo*sin)
    sinT = np.ascontiguousarray(
        np.concatenate([-sin, sin], axis=0)).astype(np.float16)

    ST = cfg.ST
    ii = np.arange(128)[:, None]
    cc = np.arange(2 * ST - 128)[None, :]
    band01 = np.where(ii <= cc - (ST - 128), 1.0, 0.0).astype(
        ml_dtypes.bfloat16)

    def wpanels(w, c):
        """[D, HPC*HD] core slice -> [128, HPC*NDB*128]: per head, per db
        block, contiguous 128 cols per partition."""
        NDB = D // 128
        sl = w[:, c * HPC * HD:(c + 1) * HPC * HD]         # [D, HPC*128]
        a = sl.reshape(NDB, 128, HPC, HD)                   # [db, p, h, c]
        a = a.transpose(1, 2, 0, 3).reshape(128, HPC * NDB * HD)
        return np.ascontiguousarray(a)

    in_maps = []
    for c in range(NC_):
        in_maps.append(dict(
            xT=xT,
            wq=wpanels(wq_eff, c),
            wk=wpanels(wk_eff, c),
            wv=wpanels(wv_eff, c),
            wo=np.ascontiguousarray(wo.astype(np.float16)),
            cosT=cosT, sinT=sinT, band=band01,
        ))
    return in_maps


_BASS_CACHE = {}


def _get_bass(cfg: Cfg):
    key = (cfg.B, cfg.S, cfg.D, cfg.HPC, cfg.NCORES)
    if key not in _BASS_CACHE:
        _BASS_CACHE[key] = build_bass(cfg)
    return _BASS_CACHE[key]


def kernel(**inputs) -> np.ndarray:
    from concourse.bass_utils import run_bass_kernel_spmd

    x = np.asarray(inputs["x"])
    B, S, D = x.shape
    cfg = Cfg(B=B, S=S, D=D, HEADS=16, NCORES=8)
    in_maps = prepare_inputs(cfg, **{k: np.asarray(v)
                                     for k, v in inputs.items()})
    nc = _get_bass(cfg)
    res = run_bass_kernel_spmd(nc, in_maps, list(range(cfg.NCORES)))
    rows = np.concatenate([res.results[c]["out_rows"]
                           for c in range(cfg.NCORES)], axis=0)
    return rows.reshape(B, S, D).astype(np.float32)


# revision 72
# speedup vs baseline: 1.0451x; 1.0271x over previous
"""Trainium2 Bass kernel for LoRA-augmented causal attention.

Reference computation (per nn_Attention_31688268710508):
  x:(B,S,D) -> q/k/v = x@W* + broadcast LoRA + shared head-offset LoRA,
  RoPE(q,k), causal softmax attention per (b,head), out-proj with wo.

Strategy (8 NeuronCores, tensor-parallel over heads):
  * All rank-8 LoRA terms folded into effective projection weights on the
    host; softmax 1/sqrt(HD) folded into Wq; RoPE pair permutation folded
    into Wq/Wk columns so RoPE is a half-partition rotation; W panels
    pre-transposed on the host so their DMAs are contiguous.
  * fp16 operands on the PE (fp32 PSUM); P/V in bf16; the causal mask is
    a 0/1 multiply after exp (bf16 has range for unmasked exp), and the
    fully-masked leading columns of diagonal blocks are trimmed from the
    scores/PV matmuls.
  * Fine-grained weave: proj 512-col sections interleave with attention
    q-tile steps at one-section lag so the PE never waits on exp/RoPE
    latency; 4 PSUM slots decouple proj matmuls from RoPE evacuation;
    softmax denominators accumulate via DVE/GpSimd adds + one ones-matmul.
  * h-major segment order: head 0's AllToAll fires at the 50% mark fully
    hidden; head 1's runs at the tail split into row-halves (small
    collectives are several times cheaper), covered by head 0's
    out-projection, with head 1's out-projection gated per row-half.
  * HWDGE queues are FIFO and block head-of-line on gated DMAs, so
    collective-gated loads are emitted only at points where nothing
    later on that queue is needed sooner.
"""

import math
import os
import sys
from contextlib import ExitStack

import ml_dtypes
import numpy as np

for _p in ("/opt/trn_rl_repo", "/root/.axon_site/_ro/trn_rl_repo"):
    if os.path.isdir(_p) and _p not in sys.path:
        sys.path.insert(0, _p)

import concourse.bass as bass  # noqa: E402
import concourse.mybir as mybir  # noqa: E402
import concourse.tile as tile  # noqa: E402
from concourse import bacc  # noqa: E402
from concourse.masks import make_identity  # noqa: E402

F32 = mybir.dt.float32
F16 = mybir.dt.float16
BF16 = mybir.dt.bfloat16
EXP = mybir.ActivationFunctionType.Exp


class Cfg:
    def __init__(self, B=2, S=2048, D=2048, HEADS=16, NCORES=8):
        self.B, self.S, self.D, self.NCORES = B, S, D, NCORES
        self.HD = 128
        self.HPC = HEADS // NCORES          # heads per core
        self.BS = B * S
        self.RPC = self.BS // NCORES        # output rows per core
        self.ST = 512                       # q tile / proj col tile
        self.SG = 512                       # x^T slab width
        self.NDB = D // 128                 # contraction blocks
        self.NHB = (HEADS * self.HD) // 128  # out-proj contraction blocks
        self.NQT = S // self.ST             # q tiles per (b,h)
        self.SPS = self.SG // self.ST       # 512-sections per slab
        assert self.HD == 128 and S % self.ST == 0 and self.RPC == self.ST


class _Emitter:
    """Holds all tiles/pools and emits proj/attention/outproj units."""

    def __init__(self, nc, tc, cfg, drams):
        self.nc, self.tc, self.cfg = nc, tc, cfg
        self.d = drams
        self.slab_cache = {}

    # ---- scope A (whole kernel) pools ----
    def open_A(self, ctx):
        tc, cfg = self.tc, self.cfg
        constp = ctx.enter_context(tc.tile_pool(name="const", bufs=1))
        self.ident = constp.tile([128, 128], BF16)
        make_identity(self.nc, self.ident)
        self.ones_sb = constp.tile([128, 128], BF16, name="ones")
        self.nc.vector.memset(self.ones_sb, 1.0)
        self.band_sb = constp.tile([128, 2 * cfg.ST - 128], BF16, name="band")

        qkv = ctx.enter_context(tc.tile_pool(name="qkv", bufs=1))
        self.QT = qkv.tile([128, cfg.HPC, cfg.BS], F16, name="QT")
        self.KT = qkv.tile([128, cfg.HPC, cfg.BS], F16, name="KT")
        self.Vhat = qkv.tile([128, cfg.HPC, cfg.B, cfg.S // 128, 128], BF16,
                             name="Vhat")

        self.ptp = ctx.enter_context(tc.tile_pool(name="ptile", bufs=4))
        self.pap = ctx.enter_context(tc.tile_pool(name="pacc", bufs=2))
        self.aotp = ctx.enter_context(tc.tile_pool(name="aot", bufs=3))
        self.rcp = ctx.enter_context(tc.tile_pool(name="rec", bufs=2))
        # head-0 half of wo, the AllToAll landing tiles and the h0 partial
        # sums are resident from the start so out-proj(h0) never waits on
        # scope-B SBUF to free
        wop0 = ctx.enter_context(tc.tile_pool(name="wo0", bufs=1))
        self.wo_h = [wop0.tile([128, cfg.NHB // cfg.HPC, cfg.D], F16,
                               name="wo_h0"), None]
        self.aip = ctx.enter_context(tc.tile_pool(name="aot_sb", bufs=1))
        self.accp = ctx.enter_context(tc.tile_pool(name="oacc", bufs=1))
        self.aot_sb = {}
        self.oacc = {}
        self.psS = ctx.enter_context(
            tc.tile_pool(name="psS", bufs=2, space="PSUM"))
        self.psO = ctx.enter_context(
            tc.tile_pool(name="psO", bufs=1, space="PSUM"))
        self.psJ = ctx.enter_context(
            tc.tile_pool(name="psJ", bufs=4, space="PSUM"))

    # ---- scope B (projection phase) pools ----
    def open_B(self, ctx):
        tc, cfg = self.tc, self.cfg
        self.xp = ctx.enter_context(tc.tile_pool(name="xslab", bufs=8))
        self.wp = ctx.enter_context(tc.tile_pool(name="wpanel", bufs=2))
        self.tbp = ctx.enter_context(tc.tile_pool(name="tables", bufs=1))
        self.rp = ctx.enter_context(tc.tile_pool(name="ropet", bufs=3))
        self.stp = ctx.enter_context(tc.tile_pool(name="vstage", bufs=2))
        self.psX = ctx.enter_context(
            tc.tile_pool(name="psX", bufs=1, space="PSUM"))
        self.cos_sb = self.tbp.tile([128, cfg.S], F16, name="cos")
        self.sin_sb = self.tbp.tile([128, cfg.S], F16, name="sin")
        self.wq_sb = {}
        self.tables_loaded = False

    def load_tables(self):
        if self.tables_loaded:
            return
        self.tables_loaded = True
        nc = self.nc
        nc.sync.dma_start(out=self.band_sb, in_=self.d["band"][:, :])
        nc.sync.dma_start(out=self.cos_sb, in_=self.d["cosT"][:, :])
        nc.sync.dma_start(out=self.sin_sb, in_=self.d["sinT"][:, :])

    def load_w_one(self, h, proj):
        if (h, proj) in self.wq_sb:
            return self.wq_sb[(h, proj)]
        nc, cfg = self.nc, self.cfg
        dram = (self.d["wq"], self.d["wk"], self.d["wv"])[proj]
        t = self.wp.tile([128, cfg.NDB, 128], F16, tag=f"w{proj}",
                         name=f"w{proj}_{h}")
        # host pre-arranged: [128p, h, db, 128c] contiguous per partition
        src = dram[:, h * cfg.NDB * 128:(h + 1) * cfg.NDB * 128]
        nc.sync.dma_start(out=t, in_=src.rearrange(
            "p (db c) -> p db c", c=128))
        self.wq_sb[(h, proj)] = t
        return t

    def slab(self, seg, b, sj):
        """x^T slab for batch b, slab index sj (SG cols), as a list of
        part-tiles each covering pdb contraction blocks.  The first slab is
        split 4 ways so the first matmuls gate on 512KB, not 2MB.

        Keyed per segment: slabs are re-loaded when a batch comes around
        again for the second head (slots cycle)."""
        nc, cfg = self.nc, self.cfg
        key = (seg, sj)
        if key in self.slab_cache:
            return self.slab_cache[key]
        nparts = 4
        pdb = cfg.NDB // nparts
        off = (b * cfg.S + sj * cfg.SG)
        src = self.d["xT"][:, off:off + cfg.SG].rearrange(
            "(db p) c -> p db c", p=128)
        parts = []
        for pi in range(nparts):
            t = self.xp.tile([128, pdb, cfg.SG], F16, tag="xsp",
                             name=f"xs{seg}_{sj}_{pi}")
            nc.sync.dma_start(out=t, in_=src[:, pi * pdb:(pi + 1) * pdb, :])
            parts.append((t, pdb))
        self.slab_cache[key] = parts
        return parts

    def rope(self, acc, dst, scol):
        """dst[f16] = RoPE(acc[PSUM f32]) using halves layout.

        cos table full 128 rows; sin table sign-folded (rows 0:64 = -sin,
        rows 64:128 = +sin).
        """
        nc, cfg = self.nc, self.cfg
        ST = cfg.ST
        t1 = self.rp.tile([128, ST], F16, tag="t1")
        nc.vector.tensor_mul(t1, acc, self.cos_sb[:, scol:scol + ST])
        t2 = self.rp.tile([128, ST], F16, tag="t2")
        nc.vector.tensor_mul(t2[0:64], acc[64:128],
                             self.sin_sb[0:64, scol:scol + ST])
        nc.vector.tensor_mul(t2[64:128], acc[0:64],
                             self.sin_sb[64:128, scol:scol + ST])
        nc.vector.tensor_add(dst, t1, t2)

    def proj_section(self, seg, b, h, sj, sec, first=False):
        """Generator: project q,k,v for 512 cols (slab sj, section sec).

        first=True sequences the Q chain before the K-weight DMA so the very
        first matmul gates on the minimum number of bytes."""
        nc, cfg = self.nc, self.cfg
        ST, NDB = cfg.ST, cfg.NDB
        wq = self.load_w_one(h, 0)
        parts = self.slab(seg, b, sj)
        if not first:
            wk = self.load_w_one(h, 1)
            wv = self.load_w_one(h, 2)
            self.load_tables()
        scol = sj * cfg.SG + sec * ST            # position within batch b
        gcol = b * cfg.S + scol                  # column in QT/KT
        c0 = sec * ST

        def xsl(db):
            t, pdb = parts[db // parts[0][1]]
            return t[:, db % pdb, c0:c0 + ST]

        accQ = self.psJ.tile([128, ST], F32, tag="pj", name=f"aQ{b}{h}{sj}{sec}")
        accK = self.psJ.tile([128, ST], F32, tag="pj", name=f"aK{b}{h}{sj}{sec}")
        if first:
            for db in range(NDB):
                nc.tensor.matmul(accQ, lhsT=wq[:, db, :], rhs=xsl(db),
                                 start=(db == 0), stop=(db == NDB - 1))
                yield
            wk = self.load_w_one(h, 1)
            wv = self.load_w_one(h, 2)
            self.load_tables()
            for db in range(NDB):
                nc.tensor.matmul(accK, lhsT=wk[:, db, :], rhs=xsl(db),
                                 start=(db == 0), stop=(db == NDB - 1))
                yield
        else:
            for db in range(NDB):
                nc.tensor.matmul(accQ, lhsT=wq[:, db, :], rhs=xsl(db),
                                 start=(db == 0), stop=(db == NDB - 1))
                yield
                nc.tensor.matmul(accK, lhsT=wk[:, db, :], rhs=xsl(db),
                                 start=(db == 0), stop=(db == NDB - 1))
                yield
        self.rope(accQ, self.QT[:, h, gcol:gcol + ST], scol)
        self.rope(accK, self.KT[:, h, gcol:gcol + ST], scol)
        accV = self.psJ.tile([128, ST], F32, tag="pj", name=f"aV{b}{h}{sj}{sec}")
        for db in range(NDB):
            nc.tensor.matmul(accV, lhsT=wv[:, db, :], rhs=xsl(db),
                             start=(db == 0), stop=(db == NDB - 1))
            yield
        vstage = self.stp.tile([128, ST], BF16, tag="vst")
        nc.scalar.copy(out=vstage, in_=accV)
        for jj in range(ST // 128):
            kblk = scol // 128 + jj
            pst = self.psX.tile([128, 128], BF16, tag="px", name="pst")
            nc.tensor.transpose(pst, vstage[:, jj * 128:(jj + 1) * 128],
                                self.ident)
            nc.scalar.copy(out=self.Vhat[:, h, b, kblk, :], in_=pst)
        yield

    def attn_qtile(self, b, h, qt):
        """Generator: attention for q-tile qt of (b,h); yields per k-block."""
        nc, cfg = self.nc, self.cfg
        S, ST = cfg.S, cfg.ST
        q0 = qt * ST
        nkb = (q0 + ST) // 128
        po = self.psO.tile([128, ST], F32, tag="po")
        pa = self.pap.tile([128, ST], BF16, tag="pa")
        for kb in range(nkb):
            d = kb * 128 - q0
            c0 = max(0, d)          # first un-masked q column of this block
            W = ST - c0
            ps = self.psS.tile([128, ST], F32, tag="ps", name=f"ps{kb%2}")
            nc.tensor.matmul(
                ps[:, c0:],
                lhsT=self.KT[:, h, b * S + kb * 128:b * S + kb * 128 + 128],
                rhs=self.QT[:, h, b * S + q0 + c0:b * S + q0 + ST],
                start=True, stop=True)
            pt = self.ptp.tile([128, ST], BF16, tag="pt")
            nc.scalar.activation(pt[:, c0:], ps[:, c0:], EXP)
            if d >= 0:
                nc.vector.tensor_mul(
                    pt[:, c0:], pt[:, c0:],
                    self.band_sb[:, ST - 128:(2 * ST - 128) - d])
            nc.tensor.matmul(po[:, c0:], lhsT=self.Vhat[:, h, b, kb, :],
                             rhs=pt[:, c0:],
                             start=(kb == 0), stop=(kb == nkb - 1))
            if kb == 0:
                nc.vector.tensor_copy(pa, pt)
            elif kb % 2 == 0:
                nc.vector.tensor_add(pa[:, c0:], pa[:, c0:], pt[:, c0:])
            else:
                nc.gpsimd.tensor_add(pa[:, c0:], pa[:, c0:], pt[:, c0:])
            yield
        pd = self.psS.tile([128, ST], F32, tag="ps", name="pd")
        nc.tensor.matmul(pd, lhsT=self.ones_sb, rhs=pa, start=True, stop=True)
        rec = self.rcp.tile([128, ST], F32, tag="rec")
        nc.vector.reciprocal_approx_fast(out=rec, in_=pd)
        aot = self.aotp.tile([128, ST], F16, tag="aot")
        nc.vector.tensor_mul(aot, po, rec)
        r0 = b * S + q0
        slot = r0 // cfg.RPC
        nv = len(self.d["a2a_ins"][h])
        hw = ST // nv
        for v in range(nv):
            nc.sync.dma_start(
                out=self.d["a2a_ins"][h][v][slot, :, :],
                in_=aot[:, v * hw:(v + 1) * hw])
        yield

    def a2a(self, h, v):
        nc, cfg = self.nc, self.cfg
        nc.gpsimd.collective_compute(
            "AllToAll",
            mybir.AluOpType.bypass,
            replica_groups=[list(range(cfg.NCORES))],
            ins=[self.d["a2a_ins"][h][v][:, :, :]],
            outs=[self.d["a2a_outs"][h][v][:, :, :]],
        )

    def load_wo(self, h):
        """wo rows for head h's blocks -> wo_h[h] tile [128, 8, D]."""
        nc, cfg = self.nc, self.cfg
        # blocks i = h, h+HPC, ... : strided view over the i dimension
        wosrc = self.d["wo"][:, :].rearrange(
            "(i hh p) c -> p i hh c", p=128, hh=cfg.HPC)
        eng = nc.sync if h == 0 else nc.scalar
        eng.dma_start(out=self.wo_h[h], in_=wosrc[:, :, h, :])

    # ---- scope C (out-projection) ----
    def open_C(self, ctx):
        tc, cfg = self.tc, self.cfg
        wop1 = ctx.enter_context(tc.tile_pool(name="wo1", bufs=1))
        self.wo_h[1] = wop1.tile([128, cfg.NHB // cfg.HPC, cfg.D], F16,
                                 name="wo_h1")
        self.obp = ctx.enter_context(tc.tile_pool(name="ob", bufs=4))
        self.load_wo(1)

    def load_aot(self, h, v):
        """aot landing tiles for (head, row-part), split by source-core
        half so the first out-proj chain gates on 256KB.  Emitted on the
        sync queue at a point where no later sync DMA is needed sooner
        (HWDGE FIFOs block head-of-line on the collective wait)."""
        nc, cfg = self.nc, self.cfg
        nv = len(self.d["a2a_outs"][h])
        hc = cfg.NCORES // 2
        tiles = []
        for g in range(2):
            a = self.aip.tile([128, hc, cfg.RPC // nv], F16,
                              tag=f"ai{h}{v}{g}", name=f"ai_sb{h}{v}{g}")
            nc.sync.dma_start(
                out=a,
                in_=self.d["a2a_outs"][h][v][g * hc:(g + 1) * hc, :, :]
                .rearrange("c p r -> p c r"))
            tiles.append(a)
        self.aot_sb[(h, v)] = tiles

    def outproj_head(self, h):
        """Generator: head h's contribution to out rows.

        Rounds of (ss, dct-pair) use 2 PSUM banks each so two rounds
        overlap in the 4-slot pj group; yields per contraction block."""
        nc, cfg = self.nc, self.cfg
        D, HPC, RPC, NHB = cfg.D, cfg.HPC, cfg.RPC, cfg.NHB
        NB = NHB // HPC
        wo = self.wo_h[h]
        nv = len(self.d["a2a_outs"][h])
        sspp = (RPC // 128) // nv          # ss strips per row-part
        NDC = D // 512
        hc = self.cfg.NCORES // 2
        for ss in range(RPC // 128):
            aotg, lr = self.aot_sb[(h, ss // sspp)], (ss % sspp) * 128

            def lhs(ib):
                return aotg[ib // hc][:, ib % hc, lr:lr + 128]

            if h == 0:
                # single-bank rounds: fits the spare psJ slot while the
                # last segment's projections still run
                for dct in range(NDC):
                    pu = self.psJ.tile([128, 512], F32, tag="pj",
                                       name=f"pu{dct}")
                    for ib in range(NB):
                        nc.tensor.matmul(
                            pu, lhsT=lhs(ib),
                            rhs=wo[:, ib, dct * 512:(dct + 1) * 512],
                            start=(ib == 0), stop=(ib == NB - 1))
                        yield
                    t = self.accp.tile([128, 512], F16,
                                       tag=f"acc{ss}_{dct}",
                                       name=f"acc{ss}_{dct}")
                    nc.scalar.copy(out=t, in_=pu)
                    self.oacc[(ss, dct)] = t
                    yield
            else:
                for half in range(NDC // 2):
                    pu = [self.psJ.tile([128, 512], F32, tag="pj",
                                        name=f"pu{half}{j}") for j in range(2)]
                    for ib in range(NB):
                        for j in range(2):
                            nc.tensor.matmul(
                                pu[j], lhsT=lhs(ib),
                                rhs=wo[:, ib, (half * 2 + j) * 512:
                                       (half * 2 + j + 1) * 512],
                                start=(ib == 0), stop=(ib == NB - 1))
                        yield
                    ob = self.obp.tile([128, 1024], F32, tag="ob")
                    for j in range(2):
                        nc.vector.tensor_add(
                            ob[:, j * 512:(j + 1) * 512], pu[j],
                            self.oacc[(ss, half * 2 + j)])
                    nc.sync.dma_start(
                        out=self.d["out_rows"][ss * 128:(ss + 1) * 128,
                                               half * 1024:(half + 1) * 1024],
                        in_=ob)
                    yield


def _weave(main_gen, feeders, drain=True):
    """Drive main_gen; after each main step, advance feeder generators so
    their progress fraction tracks the main fraction.  feeders is a list of
    [gen, total_steps, done_steps]."""
    main_steps = 0
    main_total = max(1, main_gen[1])
    gen = main_gen[0]
    while True:
        try:
            next(gen)
        except StopIteration:
            break
        main_steps += 1
        frac = min(1.0, main_steps / main_total)
        for f in feeders:
            target = int(round(f[1] * frac))
            while f[2] < target:
                if next(f[0], StopIteration) is StopIteration:
                    f[2] = f[1]
                    break
                f[2] += 1
    if drain:
        for f in feeders:
            while next(f[0], StopIteration) is not StopIteration:
                f[2] += 1


def build_bass(cfg: Cfg) -> bass.Bass:
    nc = bacc.Bacc("TRN2", target_bir_lowering=False, debug=False,
                   num_devices=cfg.NCORES)
    B, S, D = cfg.B, cfg.S, cfg.D
    HPC, HD, RPC = cfg.HPC, cfg.HD, cfg.RPC

    drams = dict(
        xT=nc.declare_dram_parameter("xT", [D, cfg.BS], F16, isOutput=False),
        wq=nc.declare_dram_parameter("wq", [128, HPC * (D // 128) * HD], F16,
                                     isOutput=False),
        wk=nc.declare_dram_parameter("wk", [128, HPC * (D // 128) * HD], F16,
                                     isOutput=False),
        wv=nc.declare_dram_parameter("wv", [128, HPC * (D // 128) * HD], F16,
                                     isOutput=False),
        wo=nc.declare_dram_parameter("wo", [cfg.NCORES * HPC * HD, D], F16,
                                     isOutput=False),
        cosT=nc.declare_dram_parameter("cosT", [128, S], F16, isOutput=False),
        sinT=nc.declare_dram_parameter("sinT", [128, S], F16, isOutput=False),
        band=nc.declare_dram_parameter("band", [128, 2 * cfg.ST - 128], BF16,
                                       isOutput=False),
        out_rows=nc.declare_dram_parameter("out_rows", [RPC, D], F32,
                                           isOutput=True),
        # head 0's AllToAll is fully overlapped -> one op (a second gpsimd
        # trigger would head-block the pa-adds on the GpSimd queue).  Head
        # 1's runs at the tail -> split in row-halves to pipeline with
        # out-projection (a 512KB collective also measures far cheaper).
        a2a_ins=[[nc.dram_tensor(f"a2a_in{h}{v}",
                                 [cfg.NCORES, HD, RPC // (1 + h)], F16)
                  for v in range(1 + h)] for h in range(HPC)],
        a2a_outs=[[nc.dram_tensor(f"a2a_out{h}{v}",
                                  [cfg.NCORES, HD, RPC // (1 + h)], F16)
                   for v in range(1 + h)] for h in range(HPC)],
    )

    # segment order: h-major (b fast) so head-0's AllToAll fires at 50%
    segs = [(0, 0), (1, 0), (0, 1), (1, 1)]

    with tile.TileContext(nc) as tc:
        with ExitStack() as ctxA:
            em = _Emitter(nc, tc, cfg, drams)
            em.open_A(ctxA)

            # proj units: (seg_idx, slab_j, section) — 512 cols each;
            # attn units: (seg_idx, qt)
            punits = [(k, sj, sec) for k in range(4)
                      for sj in range(S // cfg.SG) for sec in range(cfg.SPS)]
            aunits = [(k, qt) for k in range(4) for qt in range(cfg.NQT)]
            # pairing: A(k,qt) ⊗ P(unit qt+2 later)  (prologue: first 2 P)
            PW = 2 * cfg.NDB + cfg.NDB + 1     # yields per proj section

            def AW(qt):
                return (qt + 1) * (cfg.ST // 128) + 1

            with ExitStack() as ctxB:
                em.open_B(ctxB)
                # prologue: first proj section straight, minimal DMA gating
                for (k, sj, sec) in punits[:1]:
                    b, h = segs[k]
                    for _ in em.proj_section(k, b, h, sj, sec, first=True):
                        pass
                # main weave at lag 1; out-proj(h0) weaves into the whole
                # last segment's attention (its AllToAll landed at ~55%)
                # main weave at lag 1; the last attention q-tile runs in
                # scope C woven with the start of out-proj head 0
                for ai, (k, qt) in enumerate(aunits[:-1]):
                    b, h = segs[k]
                    feeders = []
                    pi = ai + 1
                    if pi < len(punits):
                        pk, psj, psec = punits[pi]
                        pb, ph = segs[pk]
                        feeders.append(
                            [em.proj_section(pk, pb, ph, psj, psec), PW, 0])
                    _weave((em.attn_qtile(b, h, qt), AW(qt)), feeders)
                    if k == 1 and qt == cfg.NQT - 1:
                        em.a2a(0, 0)
                        em.load_wo(0)
                # h0 landing tile: emitted after the last slab DMA so the
                # collective-gated wait can't head-block the slab stream
                em.load_aot(0, 0)

            with ExitStack() as ctxC:
                em.open_C(ctxC)
                k, qt = aunits[-1]
                b, h = segs[k]
                for _ in em.attn_qtile(b, h, qt):
                    pass
                em.a2a(1, 0)
                em.a2a(1, 1)
                # all of out-proj(h0) held to here: it is the only PE work
                # that can cover the tail AllToAll's (variable) latency
                for _ in em.outproj_head(0):
                    pass
                em.load_aot(1, 0)
                em.load_aot(1, 1)
                for _ in em.outproj_head(1):
                    pass

    nc.finalize()
    return nc


# ---------------------------------------------------------------------------
# Host side
# ---------------------------------------------------------------------------

def _rope_perm(hd):
    return np.concatenate([np.arange(0, hd, 2), np.arange(1, hd, 2)])


def prepare_inputs(cfg: Cfg, x, freq_cis, wq_base, wk_base, wv_base, head_a,
                   head_b, q_a, q_b, k_a, k_b, v_a, v_b, wo):
    """Fold LoRA + softmax scale + RoPE permutation into per-core weights."""
    B, S, D, HD, HPC, NC_ = cfg.B, cfg.S, cfg.D, cfg.HD, cfg.HPC, cfg.NCORES
    HEADS = HPC * NC_
    LORA_SCALE = 2.0
    sm = 1.0 / math.sqrt(HD)

    def fold(w_base, oa, ob):
        w = w_base.astype(np.float64).copy()
        only = LORA_SCALE * (oa.astype(np.float64) @ ob.astype(np.float64))
        hoff = LORA_SCALE * (head_a.astype(np.float64)
                             @ head_b.astype(np.float64))
        w += hoff
        w += np.tile(only, (1, HEADS))
        return w

    wq_eff = fold(wq_base, q_a, q_b) * sm
    wk_eff = fold(wk_base, k_a, k_b)
    wv_eff = fold(wv_base, v_a, v_b)

    perm = _rope_perm(HD)
    for h in range(HEADS):
        cols = h * HD + perm
        wq_eff[:, h * HD:(h + 1) * HD] = wq_eff[:, cols]
        wk_eff[:, h * HD:(h + 1) * HD] = wk_eff[:, cols]
    wq_eff = wq_eff.astype(np.float16)
    wk_eff = wk_eff.astype(np.float16)
    wv_eff = wv_eff.astype(np.float16)

    xT = np.ascontiguousarray(x.reshape(cfg.BS, D).T.astype(np.float16))

    cos = freq_cis[:S, :, 0].T.astype(np.float32)   # [64, S]
    sin = freq_cis[:S, :, 1].T.astype(np.float32)
    cosT = np.ascontiguousarray(
        np.concatenate([cos, cos], axis=0)).astype(np.float16)
    # sign-folded: top half -sin (for out_lo = lo*cos - hi*sin),
    # bottom half +sin (for out_hi = hi*cos + lo*sin)
    sinT = np.ascontiguousarray(
        np.concatenate([-sin, sin], axis=0)).astype(np.float16)

    ST = cfg.ST
    ii = np.arange(128)[:, None]
    cc = np.arange(2 * ST - 128)[None, :]
    band01 = np.where(ii <= cc - (ST - 128), 1.0, 0.0).astype(
        ml_dtypes.bfloat16)

    def wpanels(w, c):
        """[D, HPC*HD] core slice -> [128, HPC*NDB*128]: per head, per db
        block, contiguous 128 cols per partition."""
        NDB = D // 128
        sl = w[:, c * HPC * HD:(c + 1) * HPC * HD]         # [D, HPC*128]
        a = sl.reshape(NDB, 128, HPC, HD)                   # [db, p, h, c]
        a = a.transpose(1, 2, 0, 3).reshape(128, HPC * NDB * HD)
        return np.ascontiguousarray(a)

    in_maps = []
    for c in range(NC_):
        in_maps.append(dict(
            xT=xT,
            wq=wpanels(wq_eff, c),
            wk=wpanels(wk_eff, c),
            wv=wpanels(wv_eff, c),
            wo=np.ascontiguousarray(wo.astype(np.float16)),
            cosT=cosT, sinT=sinT, band=band01,
        ))
    return in_maps


_BASS_CACHE = {}


def _get_bass(cfg: Cfg):
    key = (cfg.B, cfg.S, cfg.D, cfg.HPC, cfg.NCORES)
    if key not in _BASS_CACHE:
        _BASS_CACHE[key] = build_bass(cfg)
    return _BASS_CACHE[key]


def kernel(**inputs) -> np.ndarray:
    from concourse.bass_utils import run_bass_kernel_spmd

    x = np.asarray(inputs["x"])
    B, S, D = x.shape
    cfg = Cfg(B=B, S=S, D=D, HEADS=16, NCORES=8)
    in_maps = prepare_inputs(cfg, **{k: np.asarray(v)
                                     for k, v in inputs.items()})
    nc = _get_bass(cfg)
    res = run_bass_kernel_spmd(nc, in_maps, list(range(cfg.NCORES)))
    rows = np.concatenate([res.results[c]["out_rows"]
                           for c in range(cfg.NCORES)], axis=0)
    return rows.reshape(B, S, D).astype(np.float32)
